# revision 14
# baseline (speedup 1.0000x reference)
"""Trainium2 Bass kernel for KronLinear:
    out = x @ (sum_r kron(a_r, b_r)) + bias
x: (8192, 4096) f32, a: (64,64,64), b: (64,64,64), bias: (4096,)

Sharding: 2-way over tokens x 4-way over output columns across 8 cores.
Each core:
  1. Builds its 1024-column slice of the Kronecker weight on device:
     P[(i,j),(k,l)] = sum_r a[r,i,j]*b[r,k,l] via PE matmuls (f32r) into
     PSUM, DVE-copies (casting to bf16) into an SBUF staging chunk,
     writes it CONTIGUOUSLY to a DRAM scratch, then reads back with a
     strided gather that fixes the (i,j,k,l) -> (i,k,j,l) layout into
     the resident bf16 w tiles.  (Strided small runs are on the
     HBM-read side only; strided HBM WRITES pay a per-descriptor
     completion latency that made the scatter-write variant ~25x
     slower.)
  2. Streams host-pre-tiled bf16 x^T tiles and accumulates
     out[m, n] = sum_K xT[K, m] * w[K, n] over 32 K-tiles into PSUM
     f32 (N=512 matmuls), adds bias on DVE, DMAs f32 out.
bf16 x/w halve DMA traffic and SBUF so everything double-buffers; PE
rate is identical to f32r (1 cycle/row).  PSUM accumulation stays f32.
"""
import numpy as np

RANK = 64
A1 = A2 = B1 = B2 = 64
NTOK = 8192
NCORES = 8
TH = 2            # token shards
CQ = 4            # column shards
TOK_SH = NTOK // TH          # 4096 tokens per core
COLS_SH = (A2 * B2) // CQ    # 1024 out cols per core
JPC = A2 // CQ               # 16 j-values per core
MT = TOK_SH // 128           # 32 m-tiles
KT = (A1 * B1) // 128        # 32 k-tiles

_CACHE = {}


def _build_nc(debug=False):
    import sys
    if "/opt/trn_rl_repo" not in sys.path:
        sys.path.insert(0, "/opt/trn_rl_repo")
    import concourse.tile as tile
    from concourse import bacc, mybir

    f32 = mybir.dt.float32
    f32r = mybir.dt.float32r
    bf16 = mybir.dt.bfloat16

    nc = bacc.Bacc(None, target_bir_lowering=False, debug=debug,
                   num_devices=NCORES)

    xt_d = nc.dram_tensor("xt", [MT, 128, KT * 128], bf16,
                          kind="ExternalInput")
    asel_d = nc.dram_tensor("asel", [RANK, A1 * JPC], f32r,
                            kind="ExternalInput")
    b_d = nc.dram_tensor("bb", [RANK, B1 * B2], f32r, kind="ExternalInput")
    bias_d = nc.dram_tensor("bias", [1, COLS_SH], f32, kind="ExternalInput")
    out_d = nc.dram_tensor("out", [TOK_SH, COLS_SH], f32,
                           kind="ExternalOutput")

    with tile.TileContext(nc) as tc:
        with tc.tile_pool(name="dram", bufs=1, space="DRAM") as dpool, \
             tc.tile_pool(name="const", bufs=1) as cpool, \
             tc.tile_pool(name="wres", bufs=1) as wpool, \
             tc.tile_pool(name="bld", bufs=1) as bpool, \
             tc.tile_pool(name="stg", bufs=2) as spool, \
             tc.tile_pool(name="xin", bufs=4) as xpool, \
             tc.tile_pool(name="oout", bufs=2) as opool, \
             tc.tile_pool(name="wps", bufs=4, space="PSUM") as wps_pool, \
             tc.tile_pool(name="mps", bufs=2, space="PSUM") as mps_pool:

            bias_sb = cpool.tile([128, COLS_SH], f32)
            nc.sync.dma_start(
                out=bias_sb[:],
                in_=bias_d[:, :].broadcast_to([128, COLS_SH]))

            # resident weight tiles: w_sb[t][p, (j,l)] = W[t*128+p, j*64+l]
            # with global row t*128+p == i*64 + k
            w_sb = [wpool.tile([128, COLS_SH], bf16, tag=f"w{t}",
                               name=f"w{t}")
                    for t in range(KT)]

            # P scratch in DRAM: rows (i,j), cols (k,l), written contiguous
            p_dram = dpool.tile([A1 * JPC, B1 * B2], bf16)

            # ---- Prologue: build P = sum_r a_r (x) b_r, round-trip via DRAM
            asel = bpool.tile([RANK, A1 * JPC], f32r)
            b2d = bpool.tile([RANK, B1 * B2], f32r)
            nc.sync.dma_start(out=asel[:], in_=asel_d[:, :])
            nc.sync.dma_start(out=b2d[:], in_=b_d[:, :])
            asel3 = asel[:, :].rearrange("r (i j) -> r i j", i=A1)

            # prefetch the first x tiles behind the build-factor loads
            xts_pre = []
            for mt in range(4):
                xts = xpool.tile([128, KT * 128], bf16, name=f"xp{mt}",
                                 tag="x")
                nc.sync.dma_start(out=xts[:], in_=xt_d[mt, :, :])
                xts_pre.append(xts)

            for ib in range(8):          # i-blocks of 8
                # P chunk for 8 i values: [(i8,j16)=128, (k,l)=4096] bf16
                pchunk = spool.tile([128, B1 * B2], bf16)
                for kt8 in range(8):     # (k,l)-chunks of 512
                    ps = wps_pool.tile([128, 512], f32)
                    nc.tensor.matmul(
                        ps[:],
                        asel3[:, ib * 8:(ib + 1) * 8, :],
                        b2d[:, kt8 * 512:(kt8 + 1) * 512],
                        start=True, stop=True)
                    if kt8 % 2 == 0:
                        nc.vector.tensor_copy(
                            pchunk[:, kt8 * 512:(kt8 + 1) * 512], ps[:])
                    else:
                        nc.scalar.activation(
                            pchunk[:, kt8 * 512:(kt8 + 1) * 512], ps[:],
                            mybir.ActivationFunctionType.Copy)
                # contiguous 1MB write of the chunk
                nc.sync.dma_start(
                    out=p_dram[ib * 128:(ib + 1) * 128, :],
                    in_=pchunk[:])
                # gather-read: W[i*64+k, j*64+l] = P[(i,j), (k,l)]
                # one dma per i value -> a 64-row half of a w tile
                for irel in range(8):
                    i = ib * 8 + irel
                    t, half = i // 2, i % 2
                    src = p_dram[i * JPC:(i + 1) * JPC, :] \
                        .rearrange("j (k l) -> k j l", k=B1)
                    dst = w_sb[t][half * 64:(half + 1) * 64, :] \
                        .rearrange("k (j l) -> k j l", j=JPC)
                    nc.sync.dma_start(out=dst, in_=src)

            # ---- Main loop over token tiles
            for mt in range(MT):
                if mt < 4:
                    xts = xts_pre[mt]
                else:
                    xts = xpool.tile([128, KT * 128], bf16,
                                     name=f"xm{mt}", tag="x")
                    nc.sync.dma_start(out=xts[:], in_=xt_d[mt, :, :])
                ps = mps_pool.tile([128, COLS_SH], f32)
                for kt in range(KT):
                    lt = xts[:, kt * 128:(kt + 1) * 128]
                    nc.tensor.matmul(ps[:, 0:512], lt,
                                     w_sb[kt][:, 0:512],
                                     start=(kt == 0), stop=(kt == KT - 1))
                    nc.tensor.matmul(ps[:, 512:1024], lt,
                                     w_sb[kt][:, 512:1024],
                                     start=(kt == 0), stop=(kt == KT - 1))
                osb = opool.tile([128, COLS_SH], f32)
                nc.vector.tensor_add(osb[:], ps[:], bias_sb[:])
                nc.sync.dma_start(out=out_d[mt * 128:(mt + 1) * 128, :],
                                  in_=osb[:])

    nc.compile()
    return nc


def _host_prep(x, a, b, bias):
    """Build per-core input maps."""
    import sys
    if "/opt/trn_rl_repo" not in sys.path:
        sys.path.insert(0, "/opt/trn_rl_repo")
    import ml_dtypes

    x = np.asarray(x, dtype=np.float32)
    a = np.asarray(a, dtype=np.float32)
    b = np.asarray(b, dtype=np.float32)
    bias = np.asarray(bias, dtype=np.float32)

    xb = x.astype(ml_dtypes.bfloat16)
    b2d = np.ascontiguousarray(b.reshape(RANK, B1 * B2))
    xt_by_th = []
    for th in range(TH):
        xh = xb[th * TOK_SH:(th + 1) * TOK_SH]
        # A[mt, p, kt, mm] = x[mt*128+mm, kt*128+p]
        x4 = xh.reshape(MT, 128, KT, 128)
        xt = np.ascontiguousarray(
            x4.transpose(0, 3, 2, 1)).reshape(MT, 128, KT * 128)
        xt_by_th.append(xt)
    asel_by_cq = []
    bias_by_cq = []
    for cq in range(CQ):
        asel = np.ascontiguousarray(
            a[:, :, cq * JPC:(cq + 1) * JPC].reshape(RANK, A1 * JPC))
        asel_by_cq.append(asel)
        bias_by_cq.append(np.ascontiguousarray(
            bias[cq * COLS_SH:(cq + 1) * COLS_SH].reshape(1, COLS_SH)))

    in_maps = []
    for c in range(NCORES):
        th, cq = c // CQ, c % CQ
        in_maps.append({
            "xt": xt_by_th[th],
            "asel": asel_by_cq[cq],
            "bb": b2d,
            "bias": bias_by_cq[cq],
        })
    return in_maps


def kernel(x, a, b, bias):
    import sys
    if "/opt/trn_rl_repo" not in sys.path:
        sys.path.insert(0, "/opt/trn_rl_repo")
    from concourse.bass_utils import run_bass_kernel_spmd

    if "nc" not in _CACHE:
        _CACHE["nc"] = _build_nc(debug=False)
    nc = _CACHE["nc"]

    in_maps = _host_prep(x, a, b, bias)
    res = run_bass_kernel_spmd(nc, in_maps, core_ids=list(range(NCORES)))
    out = np.empty((NTOK, A2 * B2), dtype=np.float32)
    for c in range(NCORES):
        th, cq = c // CQ, c % CQ
        out[th * TOK_SH:(th + 1) * TOK_SH,
            cq * COLS_SH:(cq + 1) * COLS_SH] = res.results[c]["out"]
    return out
